# revision 31
# baseline (speedup 1.0000x reference)
"""ContraAttention TRN2 kernel builder (v9: host-folded W2 = Wq^T Wk,
single-hop fp8 G matmul, DVE/Act route-balanced reductions, fp16 bufs).

Per-core program (core i owns query batches [16i, 16i+16)):
  G = Xq @ (Wq^T Wk) + bq Wk   (W2/bqWk folded exactly on host, fp32)
  h = Xq @ (Wq^T bk) + bq.bk    (wqbk folded on host)
  S = G @ Xk^T ; s16 = S + h (staged fp16, feeds t2v tree + transposes)
  per (a,b) 64x64 block: t2v_sum = sum_l max_m s16, v2t_sum = sum_m max_l s16
  r[a,b] = exp(ls) * (t2v_sum/cms[a] + v2t_sum/64) / 2

Big matmuls run fp8e4 DoubleRow (K=256/instr). The v2t side transposes
s16 via PE into fp16 PSUM, then per-unit drains it through a
configurable route: 'R' DVE reduce_max (1x), 'C' DVE copy @2x + SBUF
max-tree, 'A' Act stage + SBUF max-tree, 'F' fused half-copy+max @2x +
half tree; trees on DVE ('d'). NOTE: the Pool/GPSIMD engine cannot run
TensorTensor on real TRN2 (neuronxcc "Instruction engine check failed
(Pool)") - route suffix 'p' exists for modeling only, never ship it.
The t2v side is a span-merged max-tree on s16 (DVE). Epilogue runs per
4-lc half so it overlaps the main loop.

Outputs per core (same contract as v5):
  out_t2v [16, 128]  : exp(ls)/2/cms[a] * t2v_sum  at [a_local, b]
  out_v2t [2, 1024]  : exp(ls)/128 * v2t_sum at [half, ((lc*16+mt)*4+q)*2+g]
                       contributing to a_local=2*lc+g, b=mt*8+q*2+half
"""

import sys

sys.path.insert(0, "/opt/trn_rl_repo")

import concourse.bass as bass  # noqa: F401
import concourse.mybir as mybir
import concourse.tile as tile
from concourse import bacc

F32 = mybir.dt.float32
F16 = mybir.dt.float16
F8 = mybir.dt.float8e4
AF = mybir.ActivationFunctionType
AX = mybir.AxisListType
ALU = mybir.AluOpType
DR = mybir.MatmulPerfMode.DoubleRow

N_CORES = 8
NB = 128            # global batches
AB = NB // N_CORES  # 16 batches per core
L = 64              # Lq = Lk
D = 512
LQ = AB * L         # 1024 q rows per core
MK = NB * L         # 8192 k rows
NLC = LQ // 128     # 8 l-chunks
NMT = MK // 512     # 16 m-tiles
NCC = D // 128      # 4 contraction chunks
NLT = LQ // 512     # 2 l-tiles
NGRP = 16           # v2t route groups (4 units each)
HSCALE = 64.0       # host scales wqbk by this; h matmul rescales by 1/64

# default knobs (see build_kernel)
DEF_T2V_POOL = ()
DEF_V2T_ROUTES = ("R", "R", "Ad", "R", "Ad", "R", "Ad", "R",
                  "R", "Ad", "R", "Ad", "R", "Ad", "R", "R")


def _emit_tree_max(eng_l1, eng, nc, cur, scr, final_out, G, W):
    """Binary max-tree: cur [128, G, W] fp16 -> final_out [128, G] fp16.

    Intermediate levels go to scr (fp16 SBUF tile, >= G*W/2... elems).
    eng_l1 runs the first level; eng runs the rest.
    """
    off = 0
    w = W
    first = True
    while w > 1:
        half = w // 2
        in0 = cur[:, :, 0:half]
        in1 = cur[:, :, half:w]
        e = eng_l1 if first else eng
        first = False
        if half == 1:
            out = final_out.rearrange("p (g k) -> p g k", k=1)
            e.tensor_max(out, in0, in1)
        else:
            out = scr[:, off:off + G * half].rearrange(
                "p (g k) -> p g k", g=G)
            e.tensor_max(out, in0, in1)
            off += G * half
            cur = out
        w = half


def build_kernel(repeat_main=1, q8=True, span=8, t2v_pool=DEF_T2V_POOL,
                 v2t_routes=DEF_V2T_ROUTES, cast_eng="act", s16_bufs=2):
    QDT = F8 if q8 else F16
    nc = bacc.Bacc("TRN2", target_bir_lowering=False, debug=False,
                   num_devices=N_CORES)

    xqT_in = nc.dram_tensor("xqT", [128, NCC * LQ], QDT, kind="ExternalInput")
    xkT8_in = nc.dram_tensor("xkT8", [128, NCC * MK], F8,
                             kind="ExternalInput")
    w2T_in = nc.dram_tensor("w2T", [128, NCC * D], QDT, kind="ExternalInput")
    bqwk4 = nc.dram_tensor("bqwk4", [128, NCC], F32, kind="ExternalInput")
    wqbk_in = nc.dram_tensor("wqbk", [128, NCC], QDT, kind="ExternalInput")
    hconst_in = nc.dram_tensor("hconst", [128, 1], F32, kind="ExternalInput")
    mask16 = nc.dram_tensor("mask16", [AB, L], F32, kind="ExternalInput")
    ls128 = nc.dram_tensor("ls128", [128, 1], F32, kind="ExternalInput")
    ident16_in = nc.dram_tensor("ident16", [128, 128], F16,
                                kind="ExternalInput")
    sel_in = nc.dram_tensor("sel", [128, 2], F16, kind="ExternalInput")
    selb_in = nc.dram_tensor("selb", [AB, NLC * 128], F32,
                             kind="ExternalInput")

    out_t2v = nc.dram_tensor("out_t2v", [AB, 128], F32, kind="ExternalOutput")
    out_v2t = nc.dram_tensor("out_v2t", [2, NLC * NMT * 8], F32,
                             kind="ExternalOutput")

    with tile.TileContext(nc) as tc:
        with (
            tc.tile_pool(name="persist", bufs=1) as pp,
            tc.tile_pool(name="s16p", bufs=s16_bufs) as s16p,
            tc.tile_pool(name="scrp", bufs=2) as scrp,
            tc.tile_pool(name="stp", bufs=3) as stp,
            tc.tile_pool(name="vscrp", bufs=2) as vscrp,
            tc.tile_pool(name="osb", bufs=2) as osb,
            tc.tile_pool(name="pS", bufs=2, space="PSUM") as pS,
            tc.tile_pool(name="pSt", bufs=3, space="PSUM") as pSt,
            tc.tile_pool(name="pT", bufs=1, space="PSUM") as pT,
        ):
            # ---- persistent buffers ----
            # All DMAs ride the sync queue (FIFO): tiny constants first,
            # then Q-chain operands, then the 4MB xkT8 last so the Qp/G
            # matmuls need not wait behind it. Keeping the gpsimd queue
            # clear saves ~5us of Pool engine time (Pool runs v2t trees).
            xqT = pp.tile([128, NCC * LQ], QDT, tag="xqT")
            nc.sync.dma_start(xqT[:, :], xqT_in.ap())
            w2T = pp.tile([128, NCC * D], QDT, tag="w2T")
            nc.sync.dma_start(w2T[:, :], w2T_in.ap())
            bqwk_sb = pp.tile([128, NCC], F32, tag="bqwk")
            nc.sync.dma_start(bqwk_sb[:, :], bqwk4.ap())
            wqbk = pp.tile([128, NCC], QDT, tag="wqbk")
            nc.sync.dma_start(wqbk[:, :], wqbk_in.ap())
            hconst = pp.tile([128, 1], F32, tag="hconst")
            nc.sync.dma_start(hconst[:, :], hconst_in.ap())
            ident16 = pp.tile([128, 128], F16, tag="ident16")
            nc.sync.dma_start(ident16[:, :], ident16_in.ap())
            sel16 = pp.tile([128, 2], F16, tag="sel")
            nc.sync.dma_start(sel16[:, :], sel_in.ap())
            selb = pp.tile([AB, NLC * 128], F32, tag="selb")
            nc.sync.dma_start(selb[:, :], selb_in.ap())
            ls_sb = pp.tile([128, 1], F32, tag="ls")
            nc.sync.dma_start(ls_sb[:, :], ls128.ap())
            mask_sb = pp.tile([AB, L], F32, tag="mask")
            nc.sync.dma_start(mask_sb[:, :], mask16.ap())
            xkT8 = pp.tile([128, NCC * MK], F8, tag="xkT8")
            xkT8_v = xkT8[:, :].rearrange("p (cc m) -> p cc m", cc=NCC)
            xkT8_in_v = xkT8_in.ap().rearrange("p (cc m) -> p cc m", cc=NCC)
            for mt in range(NMT):
                nc.sync.dma_start(
                    xkT8_v[:, :, mt * 512:mt * 512 + 512],
                    xkT8_in_v[:, :, mt * 512:mt * 512 + 512])
            xqT_v = xqT[:, :].rearrange("p (cc l) -> p cc l", cc=NCC)
            w2T_v = w2T[:, :].rearrange("p (cc d) -> p cc d", cc=NCC)
            wqbk_v = wqbk[:, :].rearrange("p (cc o) -> p cc o", cc=NCC)

            gT8 = pp.tile([128, NCC * LQ], F8, tag="gT8")
            gT8_v = gT8[:, :].rearrange("p (cc l) -> p cc l", cc=NCC)
            h_col = pp.tile([128, NLC], F32, tag="hcol")
            recip_l = pp.tile([128, NLC], F32, tag="recipl")
            sel_sc16 = pp.tile([128, 2], F16, tag="selsc")
            t2v16 = pp.tile([128, NLC * 128], F16, tag="t2v")
            v2t16 = pp.tile([128, NLC * NMT * 8], F16, tag="v2t")

            # ---- small scalar prep ----
            expls = pp.tile([128, 1], F32, tag="expls")
            nc.scalar.activation(expls[:, :], ls_sb[:, :], AF.Exp)
            half_expls = pp.tile([128, 1], F32, tag="hexpls")
            nc.scalar.mul(half_expls[:, :], expls[:, :], 0.5)
            v2t_scale = pp.tile([128, 1], F32, tag="v2tscale")
            nc.scalar.mul(v2t_scale[:, :], expls[:, :], 1.0 / (2.0 * L))
            sel_sc32 = pp.tile([128, 2], F32, tag="selsc32")
            nc.vector.tensor_scalar_mul(sel_sc32[:, :], sel16[:, :],
                                        v2t_scale[:, 0:1])
            nc.vector.tensor_copy(sel_sc16[:, :], sel_sc32[:, :])
            msum = pp.tile([AB, 1], F32, tag="msum")
            nc.vector.reduce_sum(msum[:, :], mask_sb[:, :], axis=AX.X)
            mrec = pp.tile([AB, 1], F32, tag="mrec")
            nc.vector.reciprocal(mrec[:, :], msum[:, :])
            ps_r = pT.tile([128, NLC], F32, tag="sm", name="ps_r")
            for lc in range(NLC):
                nc.tensor.matmul(ps_r[:, lc:lc + 1],
                                 selb[:, lc * 128:lc * 128 + 128],
                                 mrec[:, 0:1],
                                 start=True, stop=True)
            # recip_l includes the exp(ls)/2 factor
            nc.vector.tensor_scalar_mul(recip_l[:, :], ps_r[:, :],
                                        half_expls[:, 0:1])

            # ---- h = (Xq @ wqbk)/HSCALE + hconst  (independent of Qp) ----
            ps_h = pT.tile([128, NLC], F32, tag="sm", name="ps_h")
            for lc in range(NLC):
                if q8:
                    for p2 in range(NCC // 2):
                        nc.tensor.matmul(
                            ps_h[:, lc:lc + 1],
                            xqT_v[:, 2 * p2:2 * p2 + 2,
                                  lc * 128:lc * 128 + 128],
                            wqbk_v[:, 2 * p2:2 * p2 + 2, 0:1],
                            start=(p2 == 0), stop=(p2 == NCC // 2 - 1),
                            perf_mode=DR)
                else:
                    for cc in range(NCC):
                        nc.tensor.matmul(
                            ps_h[:, lc:lc + 1],
                            xqT_v[:, cc, lc * 128:lc * 128 + 128],
                            wqbk_v[:, cc, 0:1],
                            start=(cc == 0), stop=(cc == NCC - 1))
            nc.scalar.activation(h_col[:, :], ps_h[:, :], AF.Identity,
                                 bias=hconst[:, 0:1], scale=1.0 / HSCALE)

            # ---- q-side: Qp per dc (fp8); G (fp8) ----
            def emit_qside(lt):
                for cc in range(NCC):
                    ps_g = pS.tile([128, 512], F32, tag="s0",
                                   name=f"ps_g_{lt}_{cc}")
                    if q8:
                        for p2 in range(NCC // 2):
                            nc.tensor.matmul(
                                ps_g[:, :],
                                w2T_v[:, 2 * p2:2 * p2 + 2,
                                      cc * 128:cc * 128 + 128],
                                xqT_v[:, 2 * p2:2 * p2 + 2,
                                      lt * 512:lt * 512 + 512],
                                start=(p2 == 0), stop=(p2 == NCC // 2 - 1),
                                perf_mode=DR)
                    else:
                        for ci in range(NCC):
                            nc.tensor.matmul(
                                ps_g[:, :],
                                w2T_v[:, ci, cc * 128:cc * 128 + 128],
                                xqT_v[:, ci, lt * 512:lt * 512 + 512],
                                start=(ci == 0), stop=(ci == NCC - 1))
                    nc.scalar.activation(
                        gT8[:, cc * LQ + lt * 512:cc * LQ + lt * 512 + 512],
                        ps_g[:, :], AF.Identity,
                        bias=bqwk_sb[:, cc:cc + 1])

            emit_qside(0)
            emit_qside(1)

            # ---- main loop ----
            SPAN = span
            for rep in range(repeat_main):
                for lc in range(NLC):
                    for pr in range(NMT // 2 // SPAN):
                        s16 = s16p.tile([128, SPAN * 1024], F16, tag="s16")
                        st16 = None
                        for sub in range(SPAN):
                            mtp = pr * SPAN + sub
                            u = lc * (NMT // 2) + mtp     # unit 0..63
                            gi = u // 4                   # route group
                            route = v2t_routes[gi]
                            ps_s = pS.tile([128, 1024], F32, tag="s0")
                            for half in range(2):
                                mt = mtp * 2 + half
                                for ccp in range(2):
                                    nc.tensor.matmul(
                                        ps_s[:, half * 512:half * 512 + 512],
                                        gT8_v[:, 2 * ccp:2 * ccp + 2,
                                              lc * 128:lc * 128 + 128],
                                        xkT8_v[:, 2 * ccp:2 * ccp + 2,
                                               mt * 512:mt * 512 + 512],
                                        start=(ccp == 0), stop=(ccp == 1),
                                        perf_mode=DR)
                            # stage S+h to fp16 (feeds t2v tree + transpose)
                            nc.scalar.activation(
                                s16[:, sub * 1024:sub * 1024 + 1024],
                                ps_s[:, :], AF.Identity,
                                bias=h_col[:, lc:lc + 1])
                            # transpose to fp16 PSUM
                            ps_t = pSt.tile([128, 1024], F16, tag="st")
                            for q in range(8):
                                nc.tensor.transpose(
                                    ps_t[:, q * 128:q * 128 + 128],
                                    s16[:, sub * 1024 + q * 128:
                                        sub * 1024 + q * 128 + 128],
                                    ident16[:, :])
                            # drain ps_t by route
                            voff = (lc * NMT + mtp * 2) * 8
                            if route[0] == "R":
                                nc.vector.reduce_max(
                                    v2t16[:, voff:voff + 16].rearrange(
                                        "p (q g) -> p q g", q=8),
                                    ps_t[:, :].rearrange(
                                        "p (q g k) -> p q g k", q=8, g=2),
                                    axis=AX.X)
                            else:
                                if u % 4 == 0:
                                    st16 = stp.tile([128, 4096], F16,
                                                    tag="st16",
                                                    name=f"st16_{lc}_{mtp}")
                                seg = st16[:, (u % 4) * 1024:
                                           (u % 4) * 1024 + 1024]
                                if route[0] == "C":
                                    nc.vector.tensor_copy(seg, ps_t[:, :])
                                else:  # "A"
                                    nc.scalar.activation(seg, ps_t[:, :],
                                                         AF.Copy)
                                if u % 4 == 3:
                                    teng = (nc.gpsimd if route[1] == "p"
                                            else nc.vector)
                                    v_scr = vscrp.tile([128, 4096], F16,
                                                       tag="vscr")
                                    _emit_tree_max(
                                        teng, teng, nc,
                                        st16[:, :].rearrange(
                                            "p (g k) -> p g k", k=L),
                                        v_scr,
                                        v2t16[:, gi * 64:gi * 64 + 64],
                                        G=64, W=L)
                        # t2v: span-merged max-tree over s16 (h included).
                        # First/last lc run as two half-trees: the first
                        # starts before the whole span is staged (prologue)
                        # and the last shortens the tail.
                        teng = (nc.gpsimd if lc in t2v_pool else nc.vector)
                        ng = SPAN * 16
                        t_scr = scrp.tile([128, SPAN * 1024], F16, tag="tscr")
                        if lc in (0, NLC - 1):
                            hw_ = SPAN * 512
                            for hh in range(2):
                                _emit_tree_max(
                                    teng, teng, nc,
                                    s16[:, hh * hw_:hh * hw_ + hw_]
                                    .rearrange("p (g k) -> p g k", k=L),
                                    t_scr[:, hh * hw_:hh * hw_ + hw_],
                                    t2v16[:, lc * 128 + hh * (ng // 2):
                                          lc * 128 + hh * (ng // 2)
                                          + ng // 2],
                                    G=ng // 2, W=L)
                        else:
                            tdst = t2v16[:, lc * 128 + pr * ng:
                                         lc * 128 + pr * ng + ng]
                            _emit_tree_max(
                                teng, teng, nc,
                                s16[:, :].rearrange("p (g k) -> p g k", k=L),
                                t_scr, tdst, G=ng, W=L)

                    # ---- epilogue halves (overlap with main loop) ----
                    if rep == repeat_main - 1 and lc in (NLC // 2,
                                                         NLC - 1):
                        hv = 0 if lc == NLC // 2 else 1
                        for lc2 in range(hv * 4, hv * 4 + 4):
                            nc.vector.tensor_scalar_mul(
                                t2v16[:, lc2 * 128:(lc2 + 1) * 128],
                                t2v16[:, lc2 * 128:(lc2 + 1) * 128],
                                recip_l[:, lc2:lc2 + 1])
                        o_sb = osb.tile([2, 1024], F32, tag="osbt",
                                        name=f"osbt_{hv}")
                        ps_o = pT.tile([2, 512], F32, tag="sm",
                                       name=f"ps_ot_{hv}")
                        nc.tensor.matmul(
                            ps_o[:, :], sel16[:, :],
                            t2v16[:, hv * 512:hv * 512 + 512],
                            start=True, stop=True)
                        nc.scalar.copy(o_sb[:, 0:512], ps_o[:, :])
                        ps_o2 = pT.tile([2, 512], F32, tag="sm",
                                        name=f"ps_ov_{hv}")
                        nc.tensor.matmul(
                            ps_o2[:, :], sel_sc16[:, :],
                            v2t16[:, hv * 512:hv * 512 + 512],
                            start=True, stop=True)
                        nc.scalar.copy(o_sb[:, 512:1024], ps_o2[:, :])
                        # o_sb[s, 0:512] -> out_t2v rows for this half
                        nc.sync.dma_start(
                            out_t2v.ap().rearrange(
                                "(lc s) b -> s lc b", s=2)
                            [:, hv * 4:hv * 4 + 4, :],
                            o_sb[:, 0:512].rearrange(
                                "p (lc b) -> p lc b", lc=4))
                        nc.sync.dma_start(
                            out_v2t.ap()[:, hv * 512:hv * 512 + 512],
                            o_sb[:, 512:1024])

    nc.compile()
    return nc


def make_host_inputs(inputs, q8=True):
    """Split full inputs into 8 per-core in_maps. inputs: dict of np arrays."""
    import numpy as np
    import ml_dtypes

    F16N = np.float16
    F8N = ml_dtypes.float8_e4m3
    QDTN = F8N if q8 else F16N

    Xq = np.ascontiguousarray(inputs["query_states"], dtype=np.float32)
    Xk = np.ascontiguousarray(inputs["key_states"], dtype=np.float32)
    mask = np.ascontiguousarray(inputs["attention_mask"], dtype=np.float32)
    Wq = np.ascontiguousarray(inputs["Wq"], dtype=np.float32)
    Wk = np.ascontiguousarray(inputs["Wk"], dtype=np.float32)
    bq = np.asarray(inputs["bq"], dtype=np.float32)
    bk = np.asarray(inputs["bk"], dtype=np.float32)
    ls = np.float32(np.asarray(inputs["logit_scale"]))

    # fold Wq into Wk: G = Xq @ (Wq^T Wk) + (bq @ Wk); both exact in fp32
    W2 = (Wq.T @ Wk).astype(np.float32)
    bqwk4 = np.ascontiguousarray((bq @ Wk).astype(np.float32)
                                 .reshape(NCC, 128).T)
    ls128 = np.full((128, 1), ls, np.float32)
    ident16 = np.eye(128, dtype=F16N)
    sel = np.zeros((128, 2), F16N)
    sel[:64, 0] = 1.0
    sel[64:, 1] = 1.0
    # selb[a, lc*128+p] = 1 iff a == 2*lc + p//64  (recip_l broadcast matmul)
    selb = np.zeros((AB, NLC * 128), np.float32)
    for lc in range(NLC):
        for p in range(128):
            selb[2 * lc + p // 64, lc * 128 + p] = 1.0

    # h projection: wqbk = HSCALE * (Wq^T @ bk); hconst = bq . bk
    wqbk_vec = (HSCALE * (Wq.T @ bk)).astype(np.float32)
    wqbk = np.ascontiguousarray(wqbk_vec.reshape(NCC, 128).T).astype(QDTN)
    hconst = np.full((128, 1), float(bq @ bk), np.float32)

    # w2T[p, cc, dout] = W2[cc*128+p, dout]
    w2T = np.ascontiguousarray(
        W2.reshape(NCC, 128, D).transpose(1, 0, 2).reshape(128, NCC * D)
    ).astype(QDTN)
    # xkT8[p, cc, m] = Xk[m, cc*128+p]
    xk2 = Xk.reshape(MK, D)
    xkT8 = np.ascontiguousarray(
        xk2.T.reshape(NCC, 128, MK).transpose(1, 0, 2).reshape(128, NCC * MK)
    ).astype(F8N)

    in_maps = []
    for i in range(N_CORES):
        xq_l = Xq[i * AB:(i + 1) * AB].reshape(LQ, D)
        xqT = np.ascontiguousarray(
            xq_l.T.reshape(NCC, 128, LQ).transpose(1, 0, 2)
            .reshape(128, NCC * LQ)).astype(QDTN)
        in_maps.append({
            "xqT": xqT,
            "xkT8": xkT8,
            "w2T": w2T,
            "bqwk4": bqwk4, "wqbk": wqbk, "hconst": hconst,
            "mask16": np.ascontiguousarray(mask[i * AB:(i + 1) * AB]),
            "ls128": ls128, "ident16": ident16, "sel": sel, "selb": selb,
        })
    return in_maps


def assemble_output(results):
    """results: list of 8 dicts with out_t2v [16,128], out_v2t [2, 1024]."""
    import numpy as np

    r = np.empty((NB, NB), np.float32)
    for i, res in enumerate(results):
        t2v = res["out_t2v"]  # [16, 128] : a_local, b
        v2t = res["out_v2t"].reshape(2, NLC, NMT, 4, 2)  # [half,lc,mt,q,g]
        # a_local = 2*lc+g ; b = mt*8 + q*2 + half
        v2t_ab = v2t.transpose(1, 4, 2, 3, 0).reshape(AB, NB)
        r[i * AB:(i + 1) * AB] = t2v + v2t_ab
    return r, np.ascontiguousarray(r.T)


# ======================= harness entry point =======================

_NC_CACHE = {}


def _get_nc():
    if "nc" not in _NC_CACHE:
        _NC_CACHE["nc"] = build_kernel()
    return _NC_CACHE["nc"]


def kernel(**inputs):
    """Full-input entry point: shards across 8 NeuronCores, runs the Bass
    kernel via PJRT SPMD, gathers per-core partial outputs, and assembles
    the full (r, r.T) result matching the reference."""
    from concourse.bass_utils import run_bass_kernel_spmd

    nc = _get_nc()
    in_maps = make_host_inputs(inputs)
    res = run_bass_kernel_spmd(nc, in_maps, core_ids=list(range(N_CORES)))
    return assemble_output(res.results)


# revision 33
# speedup vs baseline: 1.0346x; 1.0346x over previous
"""ContraAttention TRN2 kernel builder (v9: host-folded W2 = Wq^T Wk,
single-hop fp8 G matmul, DVE/Act route-balanced reductions, fp16 bufs).

Per-core program (core i owns query batches [16i, 16i+16)):
  G = Xq @ (Wq^T Wk) + bq Wk   (W2/bqWk folded exactly on host, fp32)
  h = Xq @ (Wq^T bk) + bq.bk    (wqbk folded on host)
  S = G @ Xk^T ; s16 = S + h (staged fp16, feeds t2v tree + transposes)
  per (a,b) 64x64 block: t2v_sum = sum_l max_m s16, v2t_sum = sum_m max_l s16
  r[a,b] = exp(ls) * (t2v_sum/cms[a] + v2t_sum/64) / 2

Big matmuls run fp8e4 DoubleRow (K=256/instr). The v2t side transposes
s16 via PE into fp16 PSUM, then per-unit drains it through a
configurable route: 'R' DVE reduce_max (1x), 'C' DVE copy @2x + SBUF
max-tree, 'A' Act stage + SBUF max-tree, 'F' fused half-copy+max @2x +
half tree; trees on DVE ('d'). NOTE: the Pool/GPSIMD engine cannot run
TensorTensor on real TRN2 (neuronxcc "Instruction engine check failed
(Pool)") - route suffix 'p' exists for modeling only, never ship it.
The t2v side is a span-merged max-tree on s16 (DVE). Epilogue runs per
4-lc half so it overlaps the main loop.

Outputs per core (same contract as v5):
  out_t2v [16, 128]  : exp(ls)/2/cms[a] * t2v_sum  at [a_local, b]
  out_v2t [2, 1024]  : exp(ls)/128 * v2t_sum at [half, ((lc*16+mt)*4+q)*2+g]
                       contributing to a_local=2*lc+g, b=mt*8+q*2+half
"""

import sys

sys.path.insert(0, "/opt/trn_rl_repo")

import concourse.bass as bass  # noqa: F401
import concourse.mybir as mybir
import concourse.tile as tile
from concourse import bacc

F32 = mybir.dt.float32
F16 = mybir.dt.float16
F8 = mybir.dt.float8e4
AF = mybir.ActivationFunctionType
AX = mybir.AxisListType
ALU = mybir.AluOpType
DR = mybir.MatmulPerfMode.DoubleRow

N_CORES = 8
NB = 128            # global batches
AB = NB // N_CORES  # 16 batches per core
L = 64              # Lq = Lk
D = 512
LQ = AB * L         # 1024 q rows per core
MK = NB * L         # 8192 k rows
NLC = LQ // 128     # 8 l-chunks
NMT = MK // 512     # 16 m-tiles
NCC = D // 128      # 4 contraction chunks
NLT = LQ // 512     # 2 l-tiles
NGRP = 16           # v2t route groups (4 units each)
HSCALE = 64.0       # host scales wqbk by this; h matmul rescales by 1/64

# default knobs (see build_kernel)
DEF_T2V_POOL = ()
DEF_V2T_ROUTES = ("F", "F", "H", "H", "F", "H", "H", "F",
                  "H", "H", "F", "H", "H", "F", "H", "F")


def _emit_tree_max(eng_l1, eng, nc, cur, scr, final_out, G, W):
    """Binary max-tree: cur [128, G, W] fp16 -> final_out [128, G] fp16.

    Intermediate levels go to scr (fp16 SBUF tile, >= G*W/2... elems).
    eng_l1 runs the first level; eng runs the rest.
    """
    off = 0
    w = W
    first = True
    while w > 1:
        half = w // 2
        in0 = cur[:, :, 0:half]
        in1 = cur[:, :, half:w]
        e = eng_l1 if first else eng
        first = False
        if half == 1:
            out = final_out.rearrange("p (g k) -> p g k", k=1)
            e.tensor_max(out, in0, in1)
        else:
            out = scr[:, off:off + G * half].rearrange(
                "p (g k) -> p g k", g=G)
            e.tensor_max(out, in0, in1)
            off += G * half
            cur = out
        w = half


def build_kernel(repeat_main=1, q8=True, span=8, t2v_pool=DEF_T2V_POOL,
                 v2t_routes=DEF_V2T_ROUTES, cast_eng="act", s16_bufs=2):
    QDT = F8 if q8 else F16
    nc = bacc.Bacc("TRN2", target_bir_lowering=False, debug=False,
                   num_devices=N_CORES)

    xqT_in = nc.dram_tensor("xqT", [128, NCC * LQ], QDT, kind="ExternalInput")
    xkT8_in = nc.dram_tensor("xkT8", [128, NCC * MK], F8,
                             kind="ExternalInput")
    w2T_in = nc.dram_tensor("w2T", [128, NCC * D], QDT, kind="ExternalInput")
    bqwk4 = nc.dram_tensor("bqwk4", [128, NCC], F32, kind="ExternalInput")
    wqbk_in = nc.dram_tensor("wqbk", [128, NCC], QDT, kind="ExternalInput")
    hconst_in = nc.dram_tensor("hconst", [128, 1], F32, kind="ExternalInput")
    mask16 = nc.dram_tensor("mask16", [AB, L], F32, kind="ExternalInput")
    ls128 = nc.dram_tensor("ls128", [128, 1], F32, kind="ExternalInput")
    ident16_in = nc.dram_tensor("ident16", [128, 128], F16,
                                kind="ExternalInput")
    sel_in = nc.dram_tensor("sel", [128, 2], F16, kind="ExternalInput")
    selb_in = nc.dram_tensor("selb", [AB, NLC * 128], F32,
                             kind="ExternalInput")

    out_t2v = nc.dram_tensor("out_t2v", [AB, 128], F32, kind="ExternalOutput")
    out_v2t = nc.dram_tensor("out_v2t", [2, NLC * NMT * 8], F32,
                             kind="ExternalOutput")

    with tile.TileContext(nc) as tc:
        with (
            tc.tile_pool(name="persist", bufs=1) as pp,
            tc.tile_pool(name="s16p", bufs=s16_bufs) as s16p,
            tc.tile_pool(name="scrp", bufs=2) as scrp,
            tc.tile_pool(name="stp", bufs=3) as stp,
            tc.tile_pool(name="vscrp", bufs=2) as vscrp,
            tc.tile_pool(name="osb", bufs=2) as osb,
            tc.tile_pool(name="pS", bufs=2, space="PSUM") as pS,
            tc.tile_pool(name="pSt", bufs=3, space="PSUM") as pSt,
            tc.tile_pool(name="pT", bufs=1, space="PSUM") as pT,
        ):
            # ---- persistent buffers ----
            # All DMAs ride the sync queue (FIFO): tiny constants first,
            # then Q-chain operands, then the 4MB xkT8 last so the Qp/G
            # matmuls need not wait behind it. Keeping the gpsimd queue
            # clear saves ~5us of Pool engine time (Pool runs v2t trees).
            xqT = pp.tile([128, NCC * LQ], QDT, tag="xqT")
            nc.sync.dma_start(xqT[:, :], xqT_in.ap())
            w2T = pp.tile([128, NCC * D], QDT, tag="w2T")
            nc.sync.dma_start(w2T[:, :], w2T_in.ap())
            bqwk_sb = pp.tile([128, NCC], F32, tag="bqwk")
            nc.sync.dma_start(bqwk_sb[:, :], bqwk4.ap())
            wqbk = pp.tile([128, NCC], QDT, tag="wqbk")
            nc.sync.dma_start(wqbk[:, :], wqbk_in.ap())
            hconst = pp.tile([128, 1], F32, tag="hconst")
            nc.sync.dma_start(hconst[:, :], hconst_in.ap())
            ident16 = pp.tile([128, 128], F16, tag="ident16")
            nc.sync.dma_start(ident16[:, :], ident16_in.ap())
            sel16 = pp.tile([128, 2], F16, tag="sel")
            nc.sync.dma_start(sel16[:, :], sel_in.ap())
            selb = pp.tile([AB, NLC * 128], F32, tag="selb")
            nc.sync.dma_start(selb[:, :], selb_in.ap())
            ls_sb = pp.tile([128, 1], F32, tag="ls")
            nc.sync.dma_start(ls_sb[:, :], ls128.ap())
            mask_sb = pp.tile([AB, L], F32, tag="mask")
            nc.sync.dma_start(mask_sb[:, :], mask16.ap())
            xkT8 = pp.tile([128, NCC * MK], F8, tag="xkT8")
            xkT8_v = xkT8[:, :].rearrange("p (cc m) -> p cc m", cc=NCC)
            xkT8_in_v = xkT8_in.ap().rearrange("p (cc m) -> p cc m", cc=NCC)
            for mt in range(NMT):
                nc.sync.dma_start(
                    xkT8_v[:, :, mt * 512:mt * 512 + 512],
                    xkT8_in_v[:, :, mt * 512:mt * 512 + 512])
            xqT_v = xqT[:, :].rearrange("p (cc l) -> p cc l", cc=NCC)
            w2T_v = w2T[:, :].rearrange("p (cc d) -> p cc d", cc=NCC)
            wqbk_v = wqbk[:, :].rearrange("p (cc o) -> p cc o", cc=NCC)

            gT8 = pp.tile([128, NCC * LQ], F8, tag="gT8")
            gT8_v = gT8[:, :].rearrange("p (cc l) -> p cc l", cc=NCC)
            h_col = pp.tile([128, NLC], F32, tag="hcol")
            recip_l = pp.tile([128, NLC], F32, tag="recipl")
            sel_sc16 = pp.tile([128, 2], F16, tag="selsc")
            t2v16 = pp.tile([128, NLC * 128], F16, tag="t2v")
            v2t16 = pp.tile([128, NLC * NMT * 8], F16, tag="v2t")

            # ---- small scalar prep ----
            expls = pp.tile([128, 1], F32, tag="expls")
            nc.scalar.activation(expls[:, :], ls_sb[:, :], AF.Exp)
            half_expls = pp.tile([128, 1], F32, tag="hexpls")
            nc.scalar.mul(half_expls[:, :], expls[:, :], 0.5)
            v2t_scale = pp.tile([128, 1], F32, tag="v2tscale")
            nc.scalar.mul(v2t_scale[:, :], expls[:, :], 1.0 / (2.0 * L))
            sel_sc32 = pp.tile([128, 2], F32, tag="selsc32")
            nc.vector.tensor_scalar_mul(sel_sc32[:, :], sel16[:, :],
                                        v2t_scale[:, 0:1])
            nc.vector.tensor_copy(sel_sc16[:, :], sel_sc32[:, :])
            msum = pp.tile([AB, 1], F32, tag="msum")
            nc.vector.reduce_sum(msum[:, :], mask_sb[:, :], axis=AX.X)
            mrec = pp.tile([AB, 1], F32, tag="mrec")
            nc.vector.reciprocal(mrec[:, :], msum[:, :])
            ps_r = pT.tile([128, NLC], F32, tag="sm", name="ps_r")
            for lc in range(NLC):
                nc.tensor.matmul(ps_r[:, lc:lc + 1],
                                 selb[:, lc * 128:lc * 128 + 128],
                                 mrec[:, 0:1],
                                 start=True, stop=True)
            # recip_l includes the exp(ls)/2 factor
            nc.vector.tensor_scalar_mul(recip_l[:, :], ps_r[:, :],
                                        half_expls[:, 0:1])

            # ---- h = (Xq @ wqbk)/HSCALE + hconst  (independent of Qp) ----
            ps_h = pT.tile([128, NLC], F32, tag="sm", name="ps_h")
            for lc in range(NLC):
                if q8:
                    for p2 in range(NCC // 2):
                        nc.tensor.matmul(
                            ps_h[:, lc:lc + 1],
                            xqT_v[:, 2 * p2:2 * p2 + 2,
                                  lc * 128:lc * 128 + 128],
                            wqbk_v[:, 2 * p2:2 * p2 + 2, 0:1],
                            start=(p2 == 0), stop=(p2 == NCC // 2 - 1),
                            perf_mode=DR)
                else:
                    for cc in range(NCC):
                        nc.tensor.matmul(
                            ps_h[:, lc:lc + 1],
                            xqT_v[:, cc, lc * 128:lc * 128 + 128],
                            wqbk_v[:, cc, 0:1],
                            start=(cc == 0), stop=(cc == NCC - 1))
            nc.scalar.activation(h_col[:, :], ps_h[:, :], AF.Identity,
                                 bias=hconst[:, 0:1], scale=1.0 / HSCALE)

            # ---- q-side: Qp per dc (fp8); G (fp8) ----
            def emit_qside(lt):
                for cc in range(NCC):
                    ps_g = pS.tile([128, 512], F32, tag="s0",
                                   name=f"ps_g_{lt}_{cc}")
                    if q8:
                        for p2 in range(NCC // 2):
                            nc.tensor.matmul(
                                ps_g[:, :],
                                w2T_v[:, 2 * p2:2 * p2 + 2,
                                      cc * 128:cc * 128 + 128],
                                xqT_v[:, 2 * p2:2 * p2 + 2,
                                      lt * 512:lt * 512 + 512],
                                start=(p2 == 0), stop=(p2 == NCC // 2 - 1),
                                perf_mode=DR)
                    else:
                        for ci in range(NCC):
                            nc.tensor.matmul(
                                ps_g[:, :],
                                w2T_v[:, ci, cc * 128:cc * 128 + 128],
                                xqT_v[:, ci, lt * 512:lt * 512 + 512],
                                start=(ci == 0), stop=(ci == NCC - 1))
                    nc.scalar.activation(
                        gT8[:, cc * LQ + lt * 512:cc * LQ + lt * 512 + 512],
                        ps_g[:, :], AF.Identity,
                        bias=bqwk_sb[:, cc:cc + 1])

            emit_qside(0)
            emit_qside(1)

            # ---- main loop ----
            SPAN = span
            for rep in range(repeat_main):
                for lc in range(NLC):
                    for pr in range(NMT // 2 // SPAN):
                        s16 = s16p.tile([128, SPAN * 1024], F16, tag="s16")
                        st16 = None
                        for sub in range(SPAN):
                            mtp = pr * SPAN + sub
                            u = lc * (NMT // 2) + mtp     # unit 0..63
                            gi = u // 4                   # route group
                            route = v2t_routes[gi]
                            ps_s = pS.tile([128, 1024], F32, tag="s0")
                            for half in range(2):
                                mt = mtp * 2 + half
                                for ccp in range(2):
                                    nc.tensor.matmul(
                                        ps_s[:, half * 512:half * 512 + 512],
                                        gT8_v[:, 2 * ccp:2 * ccp + 2,
                                              lc * 128:lc * 128 + 128],
                                        xkT8_v[:, 2 * ccp:2 * ccp + 2,
                                               mt * 512:mt * 512 + 512],
                                        start=(ccp == 0), stop=(ccp == 1),
                                        perf_mode=DR)
                            # stage S+h to fp16 (feeds t2v tree + transpose)
                            nc.scalar.activation(
                                s16[:, sub * 1024:sub * 1024 + 1024],
                                ps_s[:, :], AF.Identity,
                                bias=h_col[:, lc:lc + 1])
                            # transpose to fp16 PSUM
                            ps_t = pSt.tile([128, 1024], F16, tag="st")
                            for q in range(8):
                                nc.tensor.transpose(
                                    ps_t[:, q * 128:q * 128 + 128],
                                    s16[:, sub * 1024 + q * 128:
                                        sub * 1024 + q * 128 + 128],
                                    ident16[:, :])
                            # drain ps_t by route
                            voff = (lc * NMT + mtp * 2) * 8
                            if route[0] == "R":
                                nc.vector.reduce_max(
                                    v2t16[:, voff:voff + 16].rearrange(
                                        "p (q g) -> p q g", q=8),
                                    ps_t[:, :].rearrange(
                                        "p (q g k) -> p q g k", q=8, g=2),
                                    axis=AX.X)
                            elif route[0] == "F":
                                # fused half-copy: both drain ops on DVE @2x
                                if u % 4 == 0:
                                    st16 = stp.tile([128, 4096], F16,
                                                    tag="st16",
                                                    name=f"st16_{lc}_{mtp}")
                                uo = (u % 4) * 512
                                tmp = st16[:, 2048 + uo:2048 + uo + 512]
                                nc.vector.tensor_copy(
                                    tmp.rearrange("p (g k) -> p g k", g=16),
                                    ps_t[:, :].rearrange(
                                        "p (g k) -> p g k", g=16)[:, :,
                                                                 32:64])
                                nc.vector.tensor_max(
                                    st16[:, uo:uo + 512].rearrange(
                                        "p (g k) -> p g k", g=16),
                                    ps_t[:, :].rearrange(
                                        "p (g k) -> p g k", g=16)[:, :, 0:32],
                                    tmp.rearrange("p (g k) -> p g k", g=16))
                                if u % 4 == 3:
                                    v_scr = vscrp.tile([128, 4096], F16,
                                                       tag="vscr",
                                                       name=f"vscrf_{lc}_{mtp}")
                                    _emit_tree_max(
                                        nc.vector, nc.vector, nc,
                                        st16[:, 0:2048].rearrange(
                                            "p (g k) -> p g k", k=32),
                                        v_scr,
                                        v2t16[:, gi * 64:gi * 64 + 64],
                                        G=64, W=32)
                            elif route[0] == "H":
                                # half-staged drain: Act copies k[32:64),
                                # DVE fuses L1 max against PSUM k[0:32);
                                # tree (W=32) per 4-unit group on DVE.
                                if u % 4 == 0:
                                    st16 = stp.tile([128, 4096], F16,
                                                    tag="st16",
                                                    name=f"st16_{lc}_{mtp}")
                                uo = (u % 4) * 512
                                half = st16[:, 2048 + uo:2048 + uo + 512]
                                nc.scalar.activation(
                                    half.rearrange("p (g k) -> p g k", g=16),
                                    ps_t[:, :].rearrange(
                                        "p (g k) -> p g k", g=16)[:, :,
                                                                 32:64],
                                    AF.Copy)
                                nc.vector.tensor_max(
                                    st16[:, uo:uo + 512].rearrange(
                                        "p (g k) -> p g k", g=16),
                                    ps_t[:, :].rearrange(
                                        "p (g k) -> p g k", g=16)[:, :, 0:32],
                                    half.rearrange("p (g k) -> p g k", g=16))
                                if u % 4 == 3:
                                    v_scr = vscrp.tile([128, 4096], F16,
                                                       tag="vscr",
                                                       name=f"vscr_{lc}_{mtp}")
                                    _emit_tree_max(
                                        nc.vector, nc.vector, nc,
                                        st16[:, 0:2048].rearrange(
                                            "p (g k) -> p g k", k=32),
                                        v_scr,
                                        v2t16[:, gi * 64:gi * 64 + 64],
                                        G=64, W=32)
                            else:
                                if u % 4 == 0:
                                    st16 = stp.tile([128, 4096], F16,
                                                    tag="st16",
                                                    name=f"st16_{lc}_{mtp}")
                                seg = st16[:, (u % 4) * 1024:
                                           (u % 4) * 1024 + 1024]
                                if route[0] == "C":
                                    nc.vector.tensor_copy(seg, ps_t[:, :])
                                else:  # "A"
                                    nc.scalar.activation(seg, ps_t[:, :],
                                                         AF.Copy)
                                if u % 4 == 3:
                                    teng = (nc.gpsimd if route[1] == "p"
                                            else nc.vector)
                                    v_scr = vscrp.tile([128, 4096], F16,
                                                       tag="vscr")
                                    _emit_tree_max(
                                        teng, teng, nc,
                                        st16[:, :].rearrange(
                                            "p (g k) -> p g k", k=L),
                                        v_scr,
                                        v2t16[:, gi * 64:gi * 64 + 64],
                                        G=64, W=L)
                        # t2v: span-merged max-tree over s16 (h included).
                        # First/last lc run as two half-trees: the first
                        # starts before the whole span is staged (prologue)
                        # and the last shortens the tail.
                        teng = (nc.gpsimd if lc in t2v_pool else nc.vector)
                        ng = SPAN * 16
                        t_scr = scrp.tile([128, SPAN * 1024], F16, tag="tscr")
                        if lc in (0, NLC - 1):
                            hw_ = SPAN * 512
                            for hh in range(2):
                                _emit_tree_max(
                                    teng, teng, nc,
                                    s16[:, hh * hw_:hh * hw_ + hw_]
                                    .rearrange("p (g k) -> p g k", k=L),
                                    t_scr[:, hh * hw_:hh * hw_ + hw_],
                                    t2v16[:, lc * 128 + hh * (ng // 2):
                                          lc * 128 + hh * (ng // 2)
                                          + ng // 2],
                                    G=ng // 2, W=L)
                        else:
                            tdst = t2v16[:, lc * 128 + pr * ng:
                                         lc * 128 + pr * ng + ng]
                            _emit_tree_max(
                                teng, teng, nc,
                                s16[:, :].rearrange("p (g k) -> p g k", k=L),
                                t_scr, tdst, G=ng, W=L)

                    # ---- epilogue halves (overlap with main loop) ----
                    if rep == repeat_main - 1 and lc in (NLC // 2,
                                                         NLC - 1):
                        hv = 0 if lc == NLC // 2 else 1
                        for lc2 in range(hv * 4, hv * 4 + 4):
                            nc.vector.tensor_scalar_mul(
                                t2v16[:, lc2 * 128:(lc2 + 1) * 128],
                                t2v16[:, lc2 * 128:(lc2 + 1) * 128],
                                recip_l[:, lc2:lc2 + 1])
                        o_sb = osb.tile([2, 1024], F32, tag="osbt",
                                        name=f"osbt_{hv}")
                        ps_o = pT.tile([2, 512], F32, tag="sm",
                                       name=f"ps_ot_{hv}")
                        nc.tensor.matmul(
                            ps_o[:, :], sel16[:, :],
                            t2v16[:, hv * 512:hv * 512 + 512],
                            start=True, stop=True)
                        nc.scalar.copy(o_sb[:, 0:512], ps_o[:, :])
                        ps_o2 = pT.tile([2, 512], F32, tag="sm",
                                        name=f"ps_ov_{hv}")
                        nc.tensor.matmul(
                            ps_o2[:, :], sel_sc16[:, :],
                            v2t16[:, hv * 512:hv * 512 + 512],
                            start=True, stop=True)
                        nc.scalar.copy(o_sb[:, 512:1024], ps_o2[:, :])
                        # o_sb[s, 0:512] -> out_t2v rows for this half
                        nc.sync.dma_start(
                            out_t2v.ap().rearrange(
                                "(lc s) b -> s lc b", s=2)
                            [:, hv * 4:hv * 4 + 4, :],
                            o_sb[:, 0:512].rearrange(
                                "p (lc b) -> p lc b", lc=4))
                        nc.sync.dma_start(
                            out_v2t.ap()[:, hv * 512:hv * 512 + 512],
                            o_sb[:, 512:1024])

    nc.compile()
    return nc


def make_host_inputs(inputs, q8=True):
    """Split full inputs into 8 per-core in_maps. inputs: dict of np arrays."""
    import numpy as np
    import ml_dtypes

    F16N = np.float16
    F8N = ml_dtypes.float8_e4m3
    QDTN = F8N if q8 else F16N

    Xq = np.ascontiguousarray(inputs["query_states"], dtype=np.float32)
    Xk = np.ascontiguousarray(inputs["key_states"], dtype=np.float32)
    mask = np.ascontiguousarray(inputs["attention_mask"], dtype=np.float32)
    Wq = np.ascontiguousarray(inputs["Wq"], dtype=np.float32)
    Wk = np.ascontiguousarray(inputs["Wk"], dtype=np.float32)
    bq = np.asarray(inputs["bq"], dtype=np.float32)
    bk = np.asarray(inputs["bk"], dtype=np.float32)
    ls = np.float32(np.asarray(inputs["logit_scale"]))

    # fold Wq into Wk: G = Xq @ (Wq^T Wk) + (bq @ Wk); both exact in fp32
    W2 = (Wq.T @ Wk).astype(np.float32)
    bqwk4 = np.ascontiguousarray((bq @ Wk).astype(np.float32)
                                 .reshape(NCC, 128).T)
    ls128 = np.full((128, 1), ls, np.float32)
    ident16 = np.eye(128, dtype=F16N)
    sel = np.zeros((128, 2), F16N)
    sel[:64, 0] = 1.0
    sel[64:, 1] = 1.0
    # selb[a, lc*128+p] = 1 iff a == 2*lc + p//64  (recip_l broadcast matmul)
    selb = np.zeros((AB, NLC * 128), np.float32)
    for lc in range(NLC):
        for p in range(128):
            selb[2 * lc + p // 64, lc * 128 + p] = 1.0

    # h projection: wqbk = HSCALE * (Wq^T @ bk); hconst = bq . bk
    wqbk_vec = (HSCALE * (Wq.T @ bk)).astype(np.float32)
    wqbk = np.ascontiguousarray(wqbk_vec.reshape(NCC, 128).T).astype(QDTN)
    hconst = np.full((128, 1), float(bq @ bk), np.float32)

    # w2T[p, cc, dout] = W2[cc*128+p, dout]
    w2T = np.ascontiguousarray(
        W2.reshape(NCC, 128, D).transpose(1, 0, 2).reshape(128, NCC * D)
    ).astype(QDTN)
    # xkT8[p, cc, m] = Xk[m, cc*128+p]
    xk2 = Xk.reshape(MK, D)
    xkT8 = np.ascontiguousarray(
        xk2.T.reshape(NCC, 128, MK).transpose(1, 0, 2).reshape(128, NCC * MK)
    ).astype(F8N)

    in_maps = []
    for i in range(N_CORES):
        xq_l = Xq[i * AB:(i + 1) * AB].reshape(LQ, D)
        xqT = np.ascontiguousarray(
            xq_l.T.reshape(NCC, 128, LQ).transpose(1, 0, 2)
            .reshape(128, NCC * LQ)).astype(QDTN)
        in_maps.append({
            "xqT": xqT,
            "xkT8": xkT8,
            "w2T": w2T,
            "bqwk4": bqwk4, "wqbk": wqbk, "hconst": hconst,
            "mask16": np.ascontiguousarray(mask[i * AB:(i + 1) * AB]),
            "ls128": ls128, "ident16": ident16, "sel": sel, "selb": selb,
        })
    return in_maps


def assemble_output(results):
    """results: list of 8 dicts with out_t2v [16,128], out_v2t [2, 1024]."""
    import numpy as np

    r = np.empty((NB, NB), np.float32)
    for i, res in enumerate(results):
        t2v = res["out_t2v"]  # [16, 128] : a_local, b
        v2t = res["out_v2t"].reshape(2, NLC, NMT, 4, 2)  # [half,lc,mt,q,g]
        # a_local = 2*lc+g ; b = mt*8 + q*2 + half
        v2t_ab = v2t.transpose(1, 4, 2, 3, 0).reshape(AB, NB)
        r[i * AB:(i + 1) * AB] = t2v + v2t_ab
    return r, np.ascontiguousarray(r.T)


# ======================= harness entry point =======================

_NC_CACHE = {}


def _get_nc():
    if "nc" not in _NC_CACHE:
        _NC_CACHE["nc"] = build_kernel()
    return _NC_CACHE["nc"]


def kernel(**inputs):
    """Full-input entry point: shards across 8 NeuronCores, runs the Bass
    kernel via PJRT SPMD, gathers per-core partial outputs, and assembles
    the full (r, r.T) result matching the reference."""
    from concourse.bass_utils import run_bass_kernel_spmd

    nc = _get_nc()
    in_maps = make_host_inputs(inputs)
    res = run_bass_kernel_spmd(nc, in_maps, core_ids=list(range(N_CORES)))
    return assemble_output(res.results)


# revision 34
# speedup vs baseline: 1.0705x; 1.0347x over previous
"""ContraAttention TRN2 kernel builder (v9: host-folded W2 = Wq^T Wk,
single-hop fp8 G matmul, DVE/Act route-balanced reductions, fp16 bufs).

Per-core program (core i owns query batches [16i, 16i+16)):
  G = Xq @ (Wq^T Wk) + bq Wk   (W2/bqWk folded exactly on host, fp32)
  h = Xq @ (Wq^T bk) + bq.bk    (wqbk folded on host)
  S = G @ Xk^T ; s16 = S + h (staged fp16, feeds t2v tree + transposes)
  per (a,b) 64x64 block: t2v_sum = sum_l max_m s16, v2t_sum = sum_m max_l s16
  r[a,b] = exp(ls) * (t2v_sum/cms[a] + v2t_sum/64) / 2

Big matmuls run fp8e4 DoubleRow (K=256/instr). The v2t side transposes
s16 via PE into fp16 PSUM, then per-unit drains it through a
configurable route: 'R' DVE reduce_max (1x), 'C' DVE copy @2x + SBUF
max-tree, 'A' Act stage + SBUF max-tree, 'F' fused half-copy+max @2x +
half tree; trees on DVE ('d'). NOTE: the Pool/GPSIMD engine cannot run
TensorTensor on real TRN2 (neuronxcc "Instruction engine check failed
(Pool)") - route suffix 'p' exists for modeling only, never ship it.
The t2v side is a span-merged max-tree on s16 (DVE). Epilogue runs per
4-lc half so it overlaps the main loop.

Outputs per core (same contract as v5):
  out_t2v [16, 128]  : exp(ls)/2/cms[a] * t2v_sum  at [a_local, b]
  out_v2t [2, 1024]  : exp(ls)/128 * v2t_sum at [half, ((lc*16+mt)*4+q)*2+g]
                       contributing to a_local=2*lc+g, b=mt*8+q*2+half
"""

import sys

sys.path.insert(0, "/opt/trn_rl_repo")

import concourse.bass as bass  # noqa: F401
import concourse.mybir as mybir
import concourse.tile as tile
from concourse import bacc

F32 = mybir.dt.float32
F16 = mybir.dt.float16
F8 = mybir.dt.float8e4
AF = mybir.ActivationFunctionType
AX = mybir.AxisListType
ALU = mybir.AluOpType
DR = mybir.MatmulPerfMode.DoubleRow

N_CORES = 8
NB = 128            # global batches
AB = NB // N_CORES  # 16 batches per core
L = 64              # Lq = Lk
D = 512
LQ = AB * L         # 1024 q rows per core
MK = NB * L         # 8192 k rows
NLC = LQ // 128     # 8 l-chunks
NMT = MK // 512     # 16 m-tiles
NCC = D // 128      # 4 contraction chunks
NLT = LQ // 512     # 2 l-tiles
NGRP = 16           # v2t route groups (4 units each)
HSCALE = 64.0       # host scales wqbk by this; h matmul rescales by 1/64

# default knobs (see build_kernel)
DEF_T2V_POOL = ()
DEF_V2T_ROUTES = ("F", "F", "F", "H", "F", "H", "H", "F",
                  "H", "H", "F", "H", "F", "H", "H", "H")


def _emit_tree_max(eng_l1, eng, nc, cur, scr, final_out, G, W):
    """Binary max-tree: cur [128, G, W] fp16 -> final_out [128, G] fp16.

    Intermediate levels go to scr (fp16 SBUF tile, >= G*W/2... elems).
    eng_l1 runs the first level; eng runs the rest.
    """
    off = 0
    w = W
    first = True
    while w > 1:
        half = w // 2
        in0 = cur[:, :, 0:half]
        in1 = cur[:, :, half:w]
        e = eng_l1 if first else eng
        first = False
        if half == 1:
            out = final_out.rearrange("p (g k) -> p g k", k=1)
            e.tensor_max(out, in0, in1)
        else:
            out = scr[:, off:off + G * half].rearrange(
                "p (g k) -> p g k", g=G)
            e.tensor_max(out, in0, in1)
            off += G * half
            cur = out
        w = half


def build_kernel(repeat_main=1, q8=True, span=8, t2v_pool=DEF_T2V_POOL,
                 v2t_routes=DEF_V2T_ROUTES, cast_eng="act", s16_bufs=2):
    QDT = F8 if q8 else F16
    nc = bacc.Bacc("TRN2", target_bir_lowering=False, debug=False,
                   num_devices=N_CORES)

    xqT_in = nc.dram_tensor("xqT", [128, NCC * LQ], QDT, kind="ExternalInput")
    xkT8_in = nc.dram_tensor("xkT8", [128, NCC * MK], F8,
                             kind="ExternalInput")
    w2T_in = nc.dram_tensor("w2T", [128, NCC * D], QDT, kind="ExternalInput")
    bqwk4 = nc.dram_tensor("bqwk4", [128, NCC], F32, kind="ExternalInput")
    wqbk_in = nc.dram_tensor("wqbk", [128, NCC], QDT, kind="ExternalInput")
    hconst_in = nc.dram_tensor("hconst", [128, 1], F32, kind="ExternalInput")
    mask16 = nc.dram_tensor("mask16", [AB, L], F32, kind="ExternalInput")
    ls128 = nc.dram_tensor("ls128", [128, 1], F32, kind="ExternalInput")
    ident16_in = nc.dram_tensor("ident16", [128, 128], F16,
                                kind="ExternalInput")
    sel_in = nc.dram_tensor("sel", [128, 2], F16, kind="ExternalInput")
    selb_in = nc.dram_tensor("selb", [AB, NLC * 128], F32,
                             kind="ExternalInput")

    out_t2v = nc.dram_tensor("out_t2v", [AB, 128], F32, kind="ExternalOutput")
    out_v2t = nc.dram_tensor("out_v2t", [2, NLC * NMT * 8], F32,
                             kind="ExternalOutput")

    with tile.TileContext(nc) as tc:
        with (
            tc.tile_pool(name="persist", bufs=1) as pp,
            tc.tile_pool(name="s16p", bufs=s16_bufs) as s16p,
            tc.tile_pool(name="scrp", bufs=2) as scrp,
            tc.tile_pool(name="stp", bufs=3) as stp,
            tc.tile_pool(name="vscrp", bufs=2) as vscrp,
            tc.tile_pool(name="osb", bufs=2) as osb,
            tc.tile_pool(name="pS", bufs=2, space="PSUM") as pS,
            tc.tile_pool(name="pSt", bufs=3, space="PSUM") as pSt,
            tc.tile_pool(name="pT", bufs=1, space="PSUM") as pT,
        ):
            # ---- persistent buffers ----
            # All DMAs ride the sync queue (FIFO): tiny constants first,
            # then Q-chain operands, then the 4MB xkT8 last so the Qp/G
            # matmuls need not wait behind it. Keeping the gpsimd queue
            # clear saves ~5us of Pool engine time (Pool runs v2t trees).
            xqT = pp.tile([128, NCC * LQ], QDT, tag="xqT")
            nc.sync.dma_start(xqT[:, :], xqT_in.ap())
            w2T = pp.tile([128, NCC * D], QDT, tag="w2T")
            nc.sync.dma_start(w2T[:, :], w2T_in.ap())
            bqwk_sb = pp.tile([128, NCC], F32, tag="bqwk")
            nc.sync.dma_start(bqwk_sb[:, :], bqwk4.ap())
            wqbk = pp.tile([128, NCC], QDT, tag="wqbk")
            nc.sync.dma_start(wqbk[:, :], wqbk_in.ap())
            hconst = pp.tile([128, 1], F32, tag="hconst")
            nc.sync.dma_start(hconst[:, :], hconst_in.ap())
            ident16 = pp.tile([128, 128], F16, tag="ident16")
            nc.sync.dma_start(ident16[:, :], ident16_in.ap())
            sel16 = pp.tile([128, 2], F16, tag="sel")
            nc.sync.dma_start(sel16[:, :], sel_in.ap())
            selb = pp.tile([AB, NLC * 128], F32, tag="selb")
            nc.sync.dma_start(selb[:, :], selb_in.ap())
            ls_sb = pp.tile([128, 1], F32, tag="ls")
            nc.sync.dma_start(ls_sb[:, :], ls128.ap())
            mask_sb = pp.tile([AB, L], F32, tag="mask")
            nc.sync.dma_start(mask_sb[:, :], mask16.ap())
            xkT8 = pp.tile([128, NCC * MK], F8, tag="xkT8")
            xkT8_v = xkT8[:, :].rearrange("p (cc m) -> p cc m", cc=NCC)
            xkT8_in_v = xkT8_in.ap().rearrange("p (cc m) -> p cc m", cc=NCC)
            for mt in range(NMT):
                nc.sync.dma_start(
                    xkT8_v[:, :, mt * 512:mt * 512 + 512],
                    xkT8_in_v[:, :, mt * 512:mt * 512 + 512])
            xqT_v = xqT[:, :].rearrange("p (cc l) -> p cc l", cc=NCC)
            w2T_v = w2T[:, :].rearrange("p (cc d) -> p cc d", cc=NCC)
            wqbk_v = wqbk[:, :].rearrange("p (cc o) -> p cc o", cc=NCC)

            gT8 = pp.tile([128, NCC * LQ], F8, tag="gT8")
            gT8_v = gT8[:, :].rearrange("p (cc l) -> p cc l", cc=NCC)
            h_col = pp.tile([128, NLC], F32, tag="hcol")
            recip_l = pp.tile([128, NLC], F32, tag="recipl")
            sel_sc16 = pp.tile([128, 2], F16, tag="selsc")
            t2v16 = pp.tile([128, NLC * 128], F16, tag="t2v")
            v2t16 = pp.tile([128, NLC * NMT * 8], F16, tag="v2t")

            # ---- small scalar prep ----
            expls = pp.tile([128, 1], F32, tag="expls")
            nc.scalar.activation(expls[:, :], ls_sb[:, :], AF.Exp)
            half_expls = pp.tile([128, 1], F32, tag="hexpls")
            nc.scalar.mul(half_expls[:, :], expls[:, :], 0.5)
            v2t_scale = pp.tile([128, 1], F32, tag="v2tscale")
            nc.scalar.mul(v2t_scale[:, :], expls[:, :], 1.0 / (2.0 * L))
            sel_sc32 = pp.tile([128, 2], F32, tag="selsc32")
            nc.vector.tensor_scalar_mul(sel_sc32[:, :], sel16[:, :],
                                        v2t_scale[:, 0:1])
            nc.vector.tensor_copy(sel_sc16[:, :], sel_sc32[:, :])
            msum = pp.tile([AB, 1], F32, tag="msum")
            nc.vector.reduce_sum(msum[:, :], mask_sb[:, :], axis=AX.X)
            mrec = pp.tile([AB, 1], F32, tag="mrec")
            nc.vector.reciprocal(mrec[:, :], msum[:, :])
            ps_r = pT.tile([128, NLC], F32, tag="sm", name="ps_r")
            for lc in range(NLC):
                nc.tensor.matmul(ps_r[:, lc:lc + 1],
                                 selb[:, lc * 128:lc * 128 + 128],
                                 mrec[:, 0:1],
                                 start=True, stop=True)
            # recip_l includes the exp(ls)/2 factor
            nc.vector.tensor_scalar_mul(recip_l[:, :], ps_r[:, :],
                                        half_expls[:, 0:1])

            # ---- h = (Xq @ wqbk)/HSCALE + hconst  (independent of Qp) ----
            ps_h = pT.tile([128, NLC], F32, tag="sm", name="ps_h")
            for lc in range(NLC):
                if q8:
                    for p2 in range(NCC // 2):
                        nc.tensor.matmul(
                            ps_h[:, lc:lc + 1],
                            xqT_v[:, 2 * p2:2 * p2 + 2,
                                  lc * 128:lc * 128 + 128],
                            wqbk_v[:, 2 * p2:2 * p2 + 2, 0:1],
                            start=(p2 == 0), stop=(p2 == NCC // 2 - 1),
                            perf_mode=DR)
                else:
                    for cc in range(NCC):
                        nc.tensor.matmul(
                            ps_h[:, lc:lc + 1],
                            xqT_v[:, cc, lc * 128:lc * 128 + 128],
                            wqbk_v[:, cc, 0:1],
                            start=(cc == 0), stop=(cc == NCC - 1))
            nc.scalar.activation(h_col[:, :], ps_h[:, :], AF.Identity,
                                 bias=hconst[:, 0:1], scale=1.0 / HSCALE)

            # ---- q-side: Qp per dc (fp8); G (fp8) ----
            def emit_qside(lt):
                for cc in range(NCC):
                    ps_g = pS.tile([128, 512], F32, tag="s0",
                                   name=f"ps_g_{lt}_{cc}")
                    if q8:
                        for p2 in range(NCC // 2):
                            nc.tensor.matmul(
                                ps_g[:, :],
                                w2T_v[:, 2 * p2:2 * p2 + 2,
                                      cc * 128:cc * 128 + 128],
                                xqT_v[:, 2 * p2:2 * p2 + 2,
                                      lt * 512:lt * 512 + 512],
                                start=(p2 == 0), stop=(p2 == NCC // 2 - 1),
                                perf_mode=DR)
                    else:
                        for ci in range(NCC):
                            nc.tensor.matmul(
                                ps_g[:, :],
                                w2T_v[:, ci, cc * 128:cc * 128 + 128],
                                xqT_v[:, ci, lt * 512:lt * 512 + 512],
                                start=(ci == 0), stop=(ci == NCC - 1))
                    nc.scalar.activation(
                        gT8[:, cc * LQ + lt * 512:cc * LQ + lt * 512 + 512],
                        ps_g[:, :], AF.Identity,
                        bias=bqwk_sb[:, cc:cc + 1])

            emit_qside(0)
            emit_qside(1)

            # ---- main loop ----
            SPAN = span
            for rep in range(repeat_main):
                for lc in range(NLC):
                    for pr in range(NMT // 2 // SPAN):
                        s16 = s16p.tile([128, SPAN * 1024], F16, tag="s16")
                        st16 = None
                        for sub in range(SPAN):
                            mtp = pr * SPAN + sub
                            u = lc * (NMT // 2) + mtp     # unit 0..63
                            gi = u // 4                   # route group
                            route = v2t_routes[gi]
                            ps_s = pS.tile([128, 1024], F32, tag="s0")
                            for half in range(2):
                                mt = mtp * 2 + half
                                for ccp in range(2):
                                    nc.tensor.matmul(
                                        ps_s[:, half * 512:half * 512 + 512],
                                        gT8_v[:, 2 * ccp:2 * ccp + 2,
                                              lc * 128:lc * 128 + 128],
                                        xkT8_v[:, 2 * ccp:2 * ccp + 2,
                                               mt * 512:mt * 512 + 512],
                                        start=(ccp == 0), stop=(ccp == 1),
                                        perf_mode=DR)
                            # stage S+h to fp16 (feeds t2v tree + transpose)
                            nc.scalar.activation(
                                s16[:, sub * 1024:sub * 1024 + 1024],
                                ps_s[:, :], AF.Identity,
                                bias=h_col[:, lc:lc + 1])
                            # transpose to fp16 PSUM
                            ps_t = pSt.tile([128, 1024], F16, tag="st")
                            for q in range(8):
                                nc.tensor.transpose(
                                    ps_t[:, q * 128:q * 128 + 128],
                                    s16[:, sub * 1024 + q * 128:
                                        sub * 1024 + q * 128 + 128],
                                    ident16[:, :])
                            # drain ps_t by route
                            voff = (lc * NMT + mtp * 2) * 8
                            if route[0] == "R":
                                nc.vector.reduce_max(
                                    v2t16[:, voff:voff + 16].rearrange(
                                        "p (q g) -> p q g", q=8),
                                    ps_t[:, :].rearrange(
                                        "p (q g k) -> p q g k", q=8, g=2),
                                    axis=AX.X)
                            elif route[0] == "F":
                                # fused half-copy: both drain ops on DVE @2x
                                if u % 4 == 0:
                                    st16 = stp.tile([128, 4096], F16,
                                                    tag="st16",
                                                    name=f"st16_{lc}_{mtp}")
                                uo = (u % 4) * 512
                                tmp = st16[:, 2048 + uo:2048 + uo + 512]
                                nc.vector.tensor_copy(
                                    tmp.rearrange("p (g k) -> p g k", g=16),
                                    ps_t[:, :].rearrange(
                                        "p (g k) -> p g k", g=16)[:, :,
                                                                 32:64])
                                nc.vector.tensor_max(
                                    st16[:, uo:uo + 512].rearrange(
                                        "p (g k) -> p g k", g=16),
                                    ps_t[:, :].rearrange(
                                        "p (g k) -> p g k", g=16)[:, :, 0:32],
                                    tmp.rearrange("p (g k) -> p g k", g=16))
                                if u % 4 == 3:
                                    v_scr = vscrp.tile([128, 4096], F16,
                                                       tag="vscr",
                                                       name=f"vscrf_{lc}_{mtp}")
                                    _emit_tree_max(
                                        nc.vector, nc.vector, nc,
                                        st16[:, 0:2048].rearrange(
                                            "p (g k) -> p g k", k=32),
                                        v_scr,
                                        v2t16[:, gi * 64:gi * 64 + 64],
                                        G=64, W=32)
                            elif route[0] == "H":
                                # half-staged drain: Act copies k[32:64),
                                # DVE fuses L1 max against PSUM k[0:32);
                                # tree (W=32) per 4-unit group on DVE.
                                if u % 4 == 0:
                                    st16 = stp.tile([128, 4096], F16,
                                                    tag="st16",
                                                    name=f"st16_{lc}_{mtp}")
                                uo = (u % 4) * 512
                                half = st16[:, 2048 + uo:2048 + uo + 512]
                                nc.scalar.activation(
                                    half.rearrange("p (g k) -> p g k", g=16),
                                    ps_t[:, :].rearrange(
                                        "p (g k) -> p g k", g=16)[:, :,
                                                                 32:64],
                                    AF.Copy)
                                nc.vector.tensor_max(
                                    st16[:, uo:uo + 512].rearrange(
                                        "p (g k) -> p g k", g=16),
                                    ps_t[:, :].rearrange(
                                        "p (g k) -> p g k", g=16)[:, :, 0:32],
                                    half.rearrange("p (g k) -> p g k", g=16))
                                if u % 4 == 3:
                                    v_scr = vscrp.tile([128, 4096], F16,
                                                       tag="vscr",
                                                       name=f"vscr_{lc}_{mtp}")
                                    _emit_tree_max(
                                        nc.vector, nc.vector, nc,
                                        st16[:, 0:2048].rearrange(
                                            "p (g k) -> p g k", k=32),
                                        v_scr,
                                        v2t16[:, gi * 64:gi * 64 + 64],
                                        G=64, W=32)
                            else:
                                if u % 4 == 0:
                                    st16 = stp.tile([128, 4096], F16,
                                                    tag="st16",
                                                    name=f"st16_{lc}_{mtp}")
                                seg = st16[:, (u % 4) * 1024:
                                           (u % 4) * 1024 + 1024]
                                if route[0] == "C":
                                    nc.vector.tensor_copy(seg, ps_t[:, :])
                                else:  # "A"
                                    nc.scalar.activation(seg, ps_t[:, :],
                                                         AF.Copy)
                                if u % 4 == 3:
                                    teng = (nc.gpsimd if route[1] == "p"
                                            else nc.vector)
                                    v_scr = vscrp.tile([128, 4096], F16,
                                                       tag="vscr")
                                    _emit_tree_max(
                                        teng, teng, nc,
                                        st16[:, :].rearrange(
                                            "p (g k) -> p g k", k=L),
                                        v_scr,
                                        v2t16[:, gi * 64:gi * 64 + 64],
                                        G=64, W=L)
                        # t2v: span-merged max-tree over s16 (h included).
                        # First/last lc run as two half-trees: the first
                        # starts before the whole span is staged (prologue)
                        # and the last shortens the tail.
                        teng = (nc.gpsimd if lc in t2v_pool else nc.vector)
                        ng = SPAN * 16
                        t_scr = scrp.tile([128, SPAN * 1024], F16, tag="tscr")
                        if lc in (0, NLC - 1):
                            hw_ = SPAN * 512
                            for hh in range(2):
                                _emit_tree_max(
                                    teng, teng, nc,
                                    s16[:, hh * hw_:hh * hw_ + hw_]
                                    .rearrange("p (g k) -> p g k", k=L),
                                    t_scr[:, hh * hw_:hh * hw_ + hw_],
                                    t2v16[:, lc * 128 + hh * (ng // 2):
                                          lc * 128 + hh * (ng // 2)
                                          + ng // 2],
                                    G=ng // 2, W=L)
                        else:
                            tdst = t2v16[:, lc * 128 + pr * ng:
                                         lc * 128 + pr * ng + ng]
                            _emit_tree_max(
                                teng, teng, nc,
                                s16[:, :].rearrange("p (g k) -> p g k", k=L),
                                t_scr, tdst, G=ng, W=L)

                    # ---- epilogue halves (overlap with main loop) ----
                    if rep == repeat_main - 1 and lc in (NLC // 2,
                                                         NLC - 1):
                        hv = 0 if lc == NLC // 2 else 1
                        for lc2 in range(hv * 4, hv * 4 + 4):
                            nc.vector.tensor_scalar_mul(
                                t2v16[:, lc2 * 128:(lc2 + 1) * 128],
                                t2v16[:, lc2 * 128:(lc2 + 1) * 128],
                                recip_l[:, lc2:lc2 + 1])
                        o_sb = osb.tile([2, 1024], F32, tag="osbt",
                                        name=f"osbt_{hv}")
                        ps_o = pT.tile([2, 512], F32, tag="sm",
                                       name=f"ps_ot_{hv}")
                        nc.tensor.matmul(
                            ps_o[:, :], sel16[:, :],
                            t2v16[:, hv * 512:hv * 512 + 512],
                            start=True, stop=True)
                        nc.scalar.copy(o_sb[:, 0:512], ps_o[:, :])
                        ps_o2 = pT.tile([2, 512], F32, tag="sm",
                                        name=f"ps_ov_{hv}")
                        nc.tensor.matmul(
                            ps_o2[:, :], sel_sc16[:, :],
                            v2t16[:, hv * 512:hv * 512 + 512],
                            start=True, stop=True)
                        nc.scalar.copy(o_sb[:, 512:1024], ps_o2[:, :])
                        # o_sb[s, 0:512] -> out_t2v rows for this half
                        nc.sync.dma_start(
                            out_t2v.ap().rearrange(
                                "(lc s) b -> s lc b", s=2)
                            [:, hv * 4:hv * 4 + 4, :],
                            o_sb[:, 0:512].rearrange(
                                "p (lc b) -> p lc b", lc=4))
                        nc.sync.dma_start(
                            out_v2t.ap()[:, hv * 512:hv * 512 + 512],
                            o_sb[:, 512:1024])

    nc.compile()
    return nc


def make_host_inputs(inputs, q8=True):
    """Split full inputs into 8 per-core in_maps. inputs: dict of np arrays."""
    import numpy as np
    import ml_dtypes

    F16N = np.float16
    F8N = ml_dtypes.float8_e4m3
    QDTN = F8N if q8 else F16N

    Xq = np.ascontiguousarray(inputs["query_states"], dtype=np.float32)
    Xk = np.ascontiguousarray(inputs["key_states"], dtype=np.float32)
    mask = np.ascontiguousarray(inputs["attention_mask"], dtype=np.float32)
    Wq = np.ascontiguousarray(inputs["Wq"], dtype=np.float32)
    Wk = np.ascontiguousarray(inputs["Wk"], dtype=np.float32)
    bq = np.asarray(inputs["bq"], dtype=np.float32)
    bk = np.asarray(inputs["bk"], dtype=np.float32)
    ls = np.float32(np.asarray(inputs["logit_scale"]))

    # fold Wq into Wk: G = Xq @ (Wq^T Wk) + (bq @ Wk); both exact in fp32
    W2 = (Wq.T @ Wk).astype(np.float32)
    bqwk4 = np.ascontiguousarray((bq @ Wk).astype(np.float32)
                                 .reshape(NCC, 128).T)
    ls128 = np.full((128, 1), ls, np.float32)
    ident16 = np.eye(128, dtype=F16N)
    sel = np.zeros((128, 2), F16N)
    sel[:64, 0] = 1.0
    sel[64:, 1] = 1.0
    # selb[a, lc*128+p] = 1 iff a == 2*lc + p//64  (recip_l broadcast matmul)
    selb = np.zeros((AB, NLC * 128), np.float32)
    for lc in range(NLC):
        for p in range(128):
            selb[2 * lc + p // 64, lc * 128 + p] = 1.0

    # h projection: wqbk = HSCALE * (Wq^T @ bk); hconst = bq . bk
    wqbk_vec = (HSCALE * (Wq.T @ bk)).astype(np.float32)
    wqbk = np.ascontiguousarray(wqbk_vec.reshape(NCC, 128).T).astype(QDTN)
    hconst = np.full((128, 1), float(bq @ bk), np.float32)

    # w2T[p, cc, dout] = W2[cc*128+p, dout]
    w2T = np.ascontiguousarray(
        W2.reshape(NCC, 128, D).transpose(1, 0, 2).reshape(128, NCC * D)
    ).astype(QDTN)
    # xkT8[p, cc, m] = Xk[m, cc*128+p]
    xk2 = Xk.reshape(MK, D)
    xkT8 = np.ascontiguousarray(
        xk2.T.reshape(NCC, 128, MK).transpose(1, 0, 2).reshape(128, NCC * MK)
    ).astype(F8N)

    in_maps = []
    for i in range(N_CORES):
        xq_l = Xq[i * AB:(i + 1) * AB].reshape(LQ, D)
        xqT = np.ascontiguousarray(
            xq_l.T.reshape(NCC, 128, LQ).transpose(1, 0, 2)
            .reshape(128, NCC * LQ)).astype(QDTN)
        in_maps.append({
            "xqT": xqT,
            "xkT8": xkT8,
            "w2T": w2T,
            "bqwk4": bqwk4, "wqbk": wqbk, "hconst": hconst,
            "mask16": np.ascontiguousarray(mask[i * AB:(i + 1) * AB]),
            "ls128": ls128, "ident16": ident16, "sel": sel, "selb": selb,
        })
    return in_maps


def assemble_output(results):
    """results: list of 8 dicts with out_t2v [16,128], out_v2t [2, 1024]."""
    import numpy as np

    r = np.empty((NB, NB), np.float32)
    for i, res in enumerate(results):
        t2v = res["out_t2v"]  # [16, 128] : a_local, b
        v2t = res["out_v2t"].reshape(2, NLC, NMT, 4, 2)  # [half,lc,mt,q,g]
        # a_local = 2*lc+g ; b = mt*8 + q*2 + half
        v2t_ab = v2t.transpose(1, 4, 2, 3, 0).reshape(AB, NB)
        r[i * AB:(i + 1) * AB] = t2v + v2t_ab
    return r, np.ascontiguousarray(r.T)


# ======================= harness entry point =======================

_NC_CACHE = {}


def _get_nc():
    if "nc" not in _NC_CACHE:
        _NC_CACHE["nc"] = build_kernel()
    return _NC_CACHE["nc"]


def kernel(**inputs):
    """Full-input entry point: shards across 8 NeuronCores, runs the Bass
    kernel via PJRT SPMD, gathers per-core partial outputs, and assembles
    the full (r, r.T) result matching the reference."""
    from concourse.bass_utils import run_bass_kernel_spmd

    nc = _get_nc()
    in_maps = make_host_inputs(inputs)
    res = run_bass_kernel_spmd(nc, in_maps, core_ids=list(range(N_CORES)))
    return assemble_output(res.results)
